# revision 10
# baseline (speedup 1.0000x reference)
"""Biased multi-head attention kernel for 8 Trainium2 NeuronCores.

Sharding: core c = (batch b = c//2, head-group g = c%2). Each core handles one
batch element and 4 of the 8 heads (128 of the 256 channels). The output
projection (Wo) is row-parallel: each core produces a partial [QL, 256] sum and
the host adds the two partials per batch (plus bo) at gather time.

Device layout choices (per core):
  - scores are computed transposed, S_T[k, q] (k on partitions), so that
    * the additive bias streams in host-transposed [h, k, q] layout,
    * P_T needs no on-chip transpose for the P @ V matmul (contraction over k),
    * the softmax denominator falls out of the P @ V matmul via a ones-column
      appended to V.
  - softmax uses exp without max-subtraction (scores ~ N(0, 2); padded keys are
    -1e30 from the host-folded key mask, exp underflows to exactly 0).
  - q is processed in two halves of 512 so all P_T tiles for one half
    (4 heads x [1024 k, 512 q] fp32 = 8 MB) stay resident in SBUF until the
    denominators are known, then are normalized in place and written out.
"""

import sys

if "/opt/trn_rl_repo" not in sys.path:
    sys.path.insert(0, "/opt/trn_rl_repo")

import numpy as np

BS, QL, KL = 4, 1024, 1024
DQ = 256  # == K_DIM == V_DIM == Q_DIM == TOTAL
HEADS, HD = 8, 32
H = 4          # heads per core
CH = H * HD    # 128 channels per core
HALF = 512
NEG = -1.0e30
N_CORES = 8

_cached_nc = None


def _build_program():
    import concourse.bacc as bacc
    import concourse.mybir as mybir
    import concourse.tile as tile

    f32 = mybir.dt.float32
    ADD = mybir.AluOpType.add
    MULT = mybir.AluOpType.mult
    EXP = mybir.ActivationFunctionType.Exp
    SIG = mybir.ActivationFunctionType.Sigmoid
    COPY = mybir.ActivationFunctionType.Copy

    nc = bacc.Bacc("TRN2", target_bir_lowering=False, debug=False)

    qT_d = nc.dram_tensor("qT", [DQ, QL], f32, kind="ExternalInput")
    kT_d = nc.dram_tensor("kT", [DQ, KL], f32, kind="ExternalInput")
    vT_d = nc.dram_tensor("vT", [DQ, KL], f32, kind="ExternalInput")
    bias_d = nc.dram_tensor("biasT", [H, KL, QL], f32, kind="ExternalInput")
    wq_d = nc.dram_tensor("wqT", [DQ, CH], f32, kind="ExternalInput")
    wk_d = nc.dram_tensor("wkT", [DQ, CH], f32, kind="ExternalInput")
    wv_d = nc.dram_tensor("wvT", [DQ, CH], f32, kind="ExternalInput")
    wg_d = nc.dram_tensor("wgT", [DQ, CH], f32, kind="ExternalInput")
    bg_d = nc.dram_tensor("bgRow", [1, CH], f32, kind="ExternalInput")
    wo_d = nc.dram_tensor("woR", [CH, DQ], f32, kind="ExternalInput")
    id_d = nc.dram_tensor("ident", [128, 128], f32, kind="ExternalInput")
    sel_d = nc.dram_tensor("sel", [4, 4 * 128], f32, kind="ExternalInput")

    p_d = nc.dram_tensor("pOut", [H, KL, QL], f32, kind="ExternalOutput")
    o_d = nc.dram_tensor("oPart", [QL, DQ], f32, kind="ExternalOutput")

    NKT = KL // 128   # 8 k tiles
    NQT = QL // 128   # 8 q tiles

    with tile.TileContext(nc) as tc:
        with (
            tc.tile_pool(name="consts", bufs=1) as consts,
            tc.tile_pool(name="persist", bufs=1) as persist,
            tc.tile_pool(name="Ppool", bufs=5) as Ppool,
            tc.tile_pool(name="bpool", bufs=6) as bpool,
            tc.tile_pool(name="stage", bufs=3) as stage,
            tc.tile_pool(name="ps_s", bufs=4, space="PSUM") as ps_s,
            tc.tile_pool(name="ps_pv", bufs=2, space="PSUM") as ps_pv,
            tc.tile_pool(name="ps_m", bufs=2, space="PSUM") as ps_m,
        ):
            # ---- constants -------------------------------------------------
            ident = consts.tile([128, 128], f32, tag="ident", name="ident_sb")
            nc.sync.dma_start(out=ident[:], in_=id_d[:, :])
            w_sb = {}
            for nm, dram in (("wq", wq_d), ("wk", wk_d), ("wv", wv_d), ("wg", wg_d)):
                t = consts.tile([128, 2, CH], f32, tag=f"{nm}sb", name=f"{nm}_sb")
                for kc in range(2):
                    nc.sync.dma_start(out=t[:, kc, :], in_=dram[kc * 128:(kc + 1) * 128, :])
                w_sb[nm] = t
            wo_sb = consts.tile([CH, DQ], f32, tag="wosb", name="wo_sb")
            nc.sync.dma_start(out=wo_sb[:], in_=wo_d[:, :])
            bg_sb = consts.tile([1, CH], f32, tag="bgsb", name="bg_sb")
            nc.sync.dma_start(out=bg_sb[:], in_=bg_d[:, :])
            ones_row = consts.tile([1, QL], f32, tag="ones", name="ones_row")
            nc.vector.memset(ones_row[:], 1.0)
            # head-select matrices: sel[:, h, :] is [4, 128] with row h all ones
            sel = consts.tile([4, 4, 128], f32, tag="sel", name="sel_sb")
            nc.sync.dma_start(out=sel[:], in_=sel_d[:, :])

            # ---- input staging --------------------------------------------
            qT_sb = persist.tile([128, 2, QL], f32, tag="qT", name="qT_sb")
            kT_sb = persist.tile([128, 2, KL], f32, tag="kT", name="kT_sb")
            vT_sb = persist.tile([128, 2, KL], f32, tag="vT", name="vT_sb")
            for kc in range(2):
                nc.sync.dma_start(out=qT_sb[:, kc, :], in_=qT_d[kc * 128:(kc + 1) * 128, :])
                nc.sync.dma_start(out=kT_sb[:, kc, :], in_=kT_d[kc * 128:(kc + 1) * 128, :])
                nc.sync.dma_start(out=vT_sb[:, kc, :], in_=vT_d[kc * 128:(kc + 1) * 128, :])

            # ---- projections ----------------------------------------------
            # qh/kh: [128 ch, seq] transposed head projections
            qh = persist.tile([128, QL], f32, tag="qh", name="qh_sb")
            kh = persist.tile([128, KL], f32, tag="kh", name="kh_sb")
            for dst, w, src in ((qh, w_sb["wq"], qT_sb), (kh, w_sb["wk"], kT_sb)):
                for qc in range(2):
                    ps = ps_m.tile([128, 512], f32, tag="mps", name="proj_ps")
                    for kc in range(2):
                        nc.tensor.matmul(
                            ps[:], lhsT=w[:, kc, :], rhs=src[:, kc, qc * 512:(qc + 1) * 512],
                            start=(kc == 0), stop=(kc == 1))
                    nc.scalar.copy(dst[:, qc * 512:(qc + 1) * 512], ps[:])

            # vh: [k, 8 kt x (4 heads x 33)] with a ones column per head
            vh = persist.tile([128, NKT * H * 33], f32, tag="vh", name="vh_sb")
            nc.vector.memset(vh[:], 1.0)
            for kt in range(NKT):
                ps = ps_m.tile([128, CH], f32, tag="mps", name="vh_ps")
                for kc in range(2):
                    nc.tensor.matmul(
                        ps[:], lhsT=vT_sb[:, kc, kt * 128:(kt + 1) * 128], rhs=w_sb["wv"][:, kc, :],
                        start=(kc == 0), stop=(kc == 1))
                for h in range(H):
                    c0 = (kt * H + h) * 33
                    nc.scalar.copy(vh[:, c0:c0 + HD], ps[:, h * HD:(h + 1) * HD])

            # gate: sigmoid(q @ Wg^T + bg), [q, ch] layout
            gate = persist.tile([128, NQT * CH], f32, tag="gate", name="gate_sb")
            for qt in range(NQT):
                ps = ps_m.tile([128, CH], f32, tag="mps", name="gate_ps")
                for kc in range(2):
                    nc.tensor.matmul(
                        ps[:], lhsT=qT_sb[:, kc, qt * 128:(qt + 1) * 128], rhs=w_sb["wg"][:, kc, :],
                        start=(kc == 0), stop=False)
                nc.tensor.matmul(
                    ps[:], lhsT=ones_row[:, qt * 128:(qt + 1) * 128], rhs=bg_sb[:],
                    start=False, stop=True)
                nc.scalar.activation(gate[:, qt * CH:(qt + 1) * CH], ps[:], SIG)

            og = persist.tile([128, NQT * CH], f32, tag="og", name="og_sb")

            # ---- attention, one q-half at a time ---------------------------
            for half in range(2):
                q0 = half * HALF
                Ph = [Ppool.tile([128, NKT * HALF], f32, tag="P", name=f"P_{half}_{h}")
                      for h in range(H)]

                # scores + bias, exp
                for kt in range(NKT):
                    for h in range(H):
                        bt = bpool.tile([128, HALF], f32, tag="bias", name="bias_t")
                        nc.sync.dma_start(
                            out=bt[:], in_=bias_d[h, kt * 128:(kt + 1) * 128, q0:q0 + HALF])
                        ps = ps_s.tile([128, HALF], f32, tag="s", name="score_ps")
                        nc.tensor.matmul(
                            ps[:],
                            lhsT=kh[h * HD:(h + 1) * HD, kt * 128:(kt + 1) * 128],
                            rhs=qh[h * HD:(h + 1) * HD, q0:q0 + HALF],
                            start=True, stop=True, tile_position=(h * HD, 0))
                        nc.vector.tensor_tensor(
                            Ph[h][:, kt * HALF:(kt + 1) * HALF], ps[:], bt[:], ADD)
                for h in range(H):
                    nc.scalar.activation(Ph[h][:], Ph[h][:], EXP)

                # P @ [V | 1]: out rows + denominators; gather 1/denom rows
                invdr = stage.tile([4, HALF], f32, tag="invdr", name="invd_rows")
                for qt in range(4):
                    qg = half * 4 + qt
                    icols = stage.tile([128, 4], f32, tag="icols", name="invd_cols")
                    for h in range(H):
                        pso = ps_pv.tile([128, 33], f32, tag="pv", name="pv_ps")
                        for kt in range(NKT):
                            nc.tensor.matmul(
                                pso[:],
                                lhsT=Ph[h][:, kt * HALF + qt * 128: kt * HALF + (qt + 1) * 128],
                                rhs=vh[:, (kt * H + h) * 33:(kt * H + h) * 33 + 33],
                                start=(kt == 0), stop=(kt == NKT - 1))
                        nc.vector.reciprocal(icols[:, h:h + 1], pso[:, 32:33])
                        nc.scalar.activation(
                            og[:, qg * CH + h * HD: qg * CH + (h + 1) * HD],
                            pso[:, 0:HD], COPY, scale=icols[:, h:h + 1])
                    pst = ps_m.tile([4, 128], f32, tag="mps", name="invdT_ps")
                    nc.tensor.transpose(pst[:], icols[:], ident[:])
                    nc.vector.tensor_copy(invdr[:, qt * 128:(qt + 1) * 128], pst[:])

                # normalize P in place and stream out
                for h in range(H):
                    psb = ps_m.tile([128, HALF], f32, tag="mps", name="bcast_ps")
                    nc.tensor.matmul(psb[:], lhsT=sel[:, h, :], rhs=invdr[:],
                                     start=True, stop=True)
                    for kt in range(NKT):
                        sl = Ph[h][:, kt * HALF:(kt + 1) * HALF]
                        nc.vector.tensor_tensor(sl, sl, psb[:], MULT)
                        nc.sync.dma_start(
                            out=p_d[h, kt * 128:(kt + 1) * 128, q0:q0 + HALF], in_=sl)

                # gate, transpose, output projection
                for qt in range(4):
                    qg = half * 4 + qt
                    ogs = og[:, qg * CH:(qg + 1) * CH]
                    nc.vector.tensor_tensor(ogs, ogs, gate[:, qg * CH:(qg + 1) * CH], MULT)
                    pst = ps_m.tile([128, 128], f32, tag="mps", name="ogT_ps")
                    nc.tensor.transpose(pst[:], ogs, ident[:])
                    ogT = stage.tile([128, 128], f32, tag="ogT", name="ogT_sb")
                    nc.vector.tensor_copy(ogT[:], pst[:])
                    pso2 = ps_m.tile([128, DQ], f32, tag="mps", name="out_ps")
                    nc.tensor.matmul(pso2[:], lhsT=ogT[:], rhs=wo_sb[:],
                                     start=True, stop=True)
                    ot = stage.tile([128, DQ], f32, tag="ot", name="out_sb")
                    nc.scalar.copy(ot[:], pso2[:])
                    nc.sync.dma_start(out=o_d[qg * 128:(qg + 1) * 128, :], in_=ot[:])

    nc.compile()
    return nc


def _get_program():
    global _cached_nc
    if _cached_nc is None:
        _cached_nc = _build_program()
    return _cached_nc


def _host_prep(q, k, v, attn_bias, key_padding_mask, Wq, Wk, Wv, Wo, bo, Wg, bg):
    temp = np.sqrt(np.float32(HD))
    ident = np.eye(128, dtype=np.float32)
    # sel[j, h*128 + m] = 1 if j == h: selects/broadcasts row h in a K=4 matmul
    sel = np.zeros((4, 4, 128), dtype=np.float32)
    for h in range(4):
        sel[h, h, :] = 1.0
    sel = sel.reshape(4, 4 * 128)
    in_maps = []
    for c in range(N_CORES):
        b, g = divmod(c, 2)
        cols = slice(g * CH, (g + 1) * CH)
        hs = slice(g * H, (g + 1) * H)
        biasT = np.ascontiguousarray(attn_bias[b][:, :, hs].transpose(2, 1, 0))
        biasT[:, key_padding_mask[b], :] = NEG
        in_maps.append({
            "qT": np.ascontiguousarray(q[b].T),
            "kT": np.ascontiguousarray(k[b].T),
            "vT": np.ascontiguousarray(v[b].T),
            "biasT": biasT,
            "wqT": np.ascontiguousarray(Wq[cols, :].T) / temp,
            "wkT": np.ascontiguousarray(Wk[cols, :].T),
            "wvT": np.ascontiguousarray(Wv[cols, :].T),
            "wgT": np.ascontiguousarray(Wg[cols, :].T),
            "bgRow": np.ascontiguousarray(bg[cols])[None, :],
            "woR": np.ascontiguousarray(Wo[:, cols].T),
            "ident": ident,
            "sel": sel,
        })
    return in_maps


def run(q, k, v, attn_bias, key_padding_mask, Wq, Wk, Wv, Wo, bo, Wg, bg,
        trace=False):
    """Returns ((out, attn), BassKernelResults)."""
    from concourse import bass_utils

    args = [np.asarray(x) for x in
            (q, k, v, attn_bias, key_padding_mask, Wq, Wk, Wv, Wo, bo, Wg, bg)]
    (q, k, v, attn_bias, key_padding_mask, Wq, Wk, Wv, Wo, bo, Wg, bg) = args
    nc = _get_program()
    in_maps = _host_prep(q, k, v, attn_bias, key_padding_mask,
                         Wq, Wk, Wv, Wo, bo, Wg, bg)
    res = bass_utils.run_bass_kernel_spmd(
        nc, in_maps, core_ids=list(range(N_CORES)), trace=trace)

    out = np.empty((BS, QL, DQ), dtype=np.float32)
    attn = np.empty((BS, QL, KL, HEADS), dtype=np.float32)
    for b in range(BS):
        out[b] = res.results[2 * b]["oPart"] + res.results[2 * b + 1]["oPart"] \
            + bo.astype(np.float32)
        for g in range(2):
            p = res.results[2 * b + g]["pOut"]          # [H, KL, QL]
            attn[b, :, :, g * H:(g + 1) * H] = p.transpose(2, 1, 0)
    return (out, attn), res


def kernel(q, k, v, attn_bias, key_padding_mask, Wq, Wk, Wv, Wo, bo, Wg, bg):
    (out, attn), _ = run(q, k, v, attn_bias, key_padding_mask,
                         Wq, Wk, Wv, Wo, bo, Wg, bg)
    return out, attn


# revision 19
# speedup vs baseline: 1.2817x; 1.2817x over previous
"""Biased multi-head attention kernel for 8 Trainium2 NeuronCores.

Sharding: core c = (batch b = c//2, head-group g = c%2). Each core handles one
batch element and 4 of the 8 heads (128 of the 256 channels). The output
projection (Wo) is row-parallel: each core produces a partial [QL, 256] sum and
the host adds the two partials per batch (plus bo) at gather time.

Device layout choices (per core):
  - scores are computed transposed, S_T[k, q] (k on partitions), so that
    * the additive bias streams in host-transposed [h, k, q] layout,
    * P_T needs no on-chip transpose for the P @ V matmul (contraction over k),
    * the softmax denominator falls out of the P @ V matmul via a ones-column
      appended to V.
  - softmax uses exp without max-subtraction (scores ~ N(0, 2); padded keys are
    -1e30 from the host-folded key mask, exp underflows to exactly 0).
  - q is processed in two halves of 512 so all P_T tiles for one half
    (4 heads x [1024 k, 512 q] fp32 = 8 MB) stay resident in SBUF until the
    denominators are known, then are normalized in place and written out.
"""

import sys

if "/opt/trn_rl_repo" not in sys.path:
    sys.path.insert(0, "/opt/trn_rl_repo")

import numpy as np

BS, QL, KL = 4, 1024, 1024
DQ = 256  # == K_DIM == V_DIM == Q_DIM == TOTAL
HEADS, HD = 8, 32
H = 4          # heads per core
CH = H * HD    # 128 channels per core
HALF = 512
NEG = -1.0e30
N_CORES = 8

_cached_nc = None


def _build_program():
    import concourse.bacc as bacc
    import concourse.mybir as mybir
    import concourse.tile as tile

    f32 = mybir.dt.float32
    ADD = mybir.AluOpType.add
    MULT = mybir.AluOpType.mult
    EXP = mybir.ActivationFunctionType.Exp
    SIG = mybir.ActivationFunctionType.Sigmoid
    COPY = mybir.ActivationFunctionType.Copy

    nc = bacc.Bacc("TRN2", target_bir_lowering=False, debug=False)

    qT_d = nc.dram_tensor("qT", [DQ, QL], f32, kind="ExternalInput")
    kT_d = nc.dram_tensor("kT", [DQ, KL], f32, kind="ExternalInput")
    vT_d = nc.dram_tensor("vT", [DQ, KL], f32, kind="ExternalInput")
    bias_d = nc.dram_tensor("biasT", [H, KL, QL], f32, kind="ExternalInput")
    wq_d = nc.dram_tensor("wqT", [DQ, CH], f32, kind="ExternalInput")
    wk_d = nc.dram_tensor("wkT", [DQ, CH], f32, kind="ExternalInput")
    wv_d = nc.dram_tensor("wvT", [DQ, CH], f32, kind="ExternalInput")
    wg_d = nc.dram_tensor("wgT", [DQ, CH], f32, kind="ExternalInput")
    bg_d = nc.dram_tensor("bgCol", [CH, 1], f32, kind="ExternalInput")
    wo_d = nc.dram_tensor("woR", [CH, DQ], f32, kind="ExternalInput")
    sel_d = nc.dram_tensor("sel", [4, 4 * 128], f32, kind="ExternalInput")
    sel32_d = nc.dram_tensor("sel32", [4, 128], f32, kind="ExternalInput")

    p_d = nc.dram_tensor("pOut", [H, KL, QL], f32, kind="ExternalOutput")
    o_d = nc.dram_tensor("oPart", [QL, DQ], f32, kind="ExternalOutput")

    NKT = KL // 128   # 8 k tiles
    NQT = QL // 128   # 8 q tiles

    with tile.TileContext(nc) as tc:
        with (
            tc.tile_pool(name="consts", bufs=1) as consts,
            tc.tile_pool(name="persist", bufs=1) as persist,
            tc.tile_pool(name="Ppool", bufs=5) as Ppool,
            tc.tile_pool(name="bpool", bufs=6) as bpool,
            tc.tile_pool(name="stage", bufs=3) as stage,
            tc.tile_pool(name="ps_s", bufs=4, space="PSUM") as ps_s,
            tc.tile_pool(name="ps_pv", bufs=2, space="PSUM") as ps_pv,
            tc.tile_pool(name="ps_m", bufs=2, space="PSUM") as ps_m,
        ):
            # ---- constants -------------------------------------------------
            w_sb = {}
            for nm, dram in (("wq", wq_d), ("wk", wk_d), ("wv", wv_d), ("wg", wg_d)):
                t = consts.tile([128, 2, CH], f32, tag=f"{nm}sb", name=f"{nm}_sb")
                for kc in range(2):
                    nc.sync.dma_start(out=t[:, kc, :], in_=dram[kc * 128:(kc + 1) * 128, :])
                w_sb[nm] = t
            wo_sb = consts.tile([CH, DQ], f32, tag="wosb", name="wo_sb")
            nc.sync.dma_start(out=wo_sb[:], in_=wo_d[:, :])
            bg_sb = consts.tile([CH, 1], f32, tag="bgsb", name="bg_sb")
            nc.sync.dma_start(out=bg_sb[:], in_=bg_d[:, :])
            # head-select matrices: sel[:, h, :] is [4, 128] with row h all ones;
            # sel32[j, m] = 1 if m // 32 == j (per-head broadcast for og rows)
            sel = consts.tile([4, 4, 128], f32, tag="sel", name="sel_sb")
            nc.sync.dma_start(out=sel[:], in_=sel_d[:, :])
            sel32 = consts.tile([4, 128], f32, tag="sel32", name="sel32_sb")
            nc.sync.dma_start(out=sel32[:], in_=sel32_d[:, :])

            # ---- input staging --------------------------------------------
            qT_sb = persist.tile([128, 2, QL], f32, tag="qT", name="qT_sb")
            kT_sb = persist.tile([128, 2, KL], f32, tag="kT", name="kT_sb")
            vT_sb = persist.tile([128, 2, KL], f32, tag="vT", name="vT_sb")
            for kc in range(2):
                nc.sync.dma_start(out=qT_sb[:, kc, :], in_=qT_d[kc * 128:(kc + 1) * 128, :])
                nc.sync.dma_start(out=kT_sb[:, kc, :], in_=kT_d[kc * 128:(kc + 1) * 128, :])
                nc.sync.dma_start(out=vT_sb[:, kc, :], in_=vT_d[kc * 128:(kc + 1) * 128, :])

            # ---- projections ----------------------------------------------
            # qh/kh: [128 ch, seq] transposed head projections
            qh = persist.tile([128, QL], f32, tag="qh", name="qh_sb")
            kh = persist.tile([128, KL], f32, tag="kh", name="kh_sb")
            for dst, w, src in ((qh, w_sb["wq"], qT_sb), (kh, w_sb["wk"], kT_sb)):
                for qc in range(2):
                    ps = ps_m.tile([128, 512], f32, tag="mps", name="proj_ps")
                    for kc in range(2):
                        nc.tensor.matmul(
                            ps[:], lhsT=w[:, kc, :], rhs=src[:, kc, qc * 512:(qc + 1) * 512],
                            start=(kc == 0), stop=(kc == 1))
                    nc.scalar.copy(dst[:, qc * 512:(qc + 1) * 512], ps[:])

            # vh: [k, 8 kt x (4 heads x 33)] with a ones column per head
            vh = persist.tile([128, NKT * H * 33], f32, tag="vh", name="vh_sb")
            nc.vector.memset(vh[:], 1.0)
            for kt in range(NKT):
                ps = ps_m.tile([128, CH], f32, tag="mps", name="vh_ps")
                for kc in range(2):
                    nc.tensor.matmul(
                        ps[:], lhsT=vT_sb[:, kc, kt * 128:(kt + 1) * 128], rhs=w_sb["wv"][:, kc, :],
                        start=(kc == 0), stop=(kc == 1))
                for h in range(H):
                    c0 = (kt * H + h) * 33
                    nc.scalar.copy(vh[:, c0:c0 + HD], ps[:, h * HD:(h + 1) * HD])

            # gate: sigmoid(Wg @ q^T + bg), transposed [ch, q] layout
            gate = persist.tile([128, QL], f32, tag="gate", name="gate_sb")
            for qc in range(2):
                ps = ps_m.tile([128, 512], f32, tag="mps", name="gate_ps")
                for kc in range(2):
                    nc.tensor.matmul(
                        ps[:], lhsT=w_sb["wg"][:, kc, :], rhs=qT_sb[:, kc, qc * 512:(qc + 1) * 512],
                        start=(kc == 0), stop=(kc == 1))
                nc.scalar.activation(gate[:, qc * 512:(qc + 1) * 512], ps[:], SIG,
                                     bias=bg_sb[:, 0:1])

            # gated attention output, transposed [ch, q] layout
            og = persist.tile([128, QL], f32, tag="og", name="og_sb")

            # ---- attention, one q-half at a time ---------------------------
            for half in range(2):
                q0 = half * HALF
                Ph = [Ppool.tile([128, NKT * HALF], f32, tag="P", name=f"P_{half}_{h}")
                      for h in range(H)]

                # scores + bias, exp
                for kt in range(NKT):
                    bt = bpool.tile([128, H, HALF], f32, tag="bias", name="bias_t")
                    nc.sync.dma_start(
                        out=bt[:],
                        in_=bias_d[0:H, kt * 128:(kt + 1) * 128, q0:q0 + HALF]
                        .rearrange("h k q -> k h q"))
                    for h in range(H):
                        ps = ps_s.tile([128, HALF], f32, tag="s", name="score_ps")
                        nc.tensor.matmul(
                            ps[:],
                            lhsT=kh[h * HD:(h + 1) * HD, kt * 128:(kt + 1) * 128],
                            rhs=qh[h * HD:(h + 1) * HD, q0:q0 + HALF],
                            start=True, stop=True, tile_position=(h * HD, 0))
                        nc.vector.tensor_tensor(
                            Ph[h][:, kt * HALF:(kt + 1) * HALF], ps[:],
                            bt[:, h, :], ADD)
                for h in range(H):
                    nc.scalar.activation(Ph[h][:], Ph[h][:], EXP)

                # [V | 1]^T @ P: og rows (transposed) + denominator row
                invdr = stage.tile([4, HALF], f32, tag="invdr", name="invd_rows")
                for h in range(H):
                    pso = ps_pv.tile([33, HALF], f32, tag="pv", name="pv_ps")
                    for kt in range(NKT):
                        nc.tensor.matmul(
                            pso[:],
                            lhsT=vh[:, (kt * H + h) * 33:(kt * H + h) * 33 + 33],
                            rhs=Ph[h][:, kt * HALF:(kt + 1) * HALF],
                            start=(kt == 0), stop=(kt == NKT - 1))
                    # 1/denominator: row 32 (partition-32-aligned), then DMA-shift
                    st33 = stage.tile([33, HALF], f32, tag="st33", name="st33_sb")
                    nc.vector.reciprocal(st33[32:33, :], pso[32:33, :])
                    nc.sync.dma_start(out=invdr[h:h + 1, :], in_=st33[32:33, :])
                    # og rows: copy out of psum, DMA-shift into og[ch, q]
                    ogst = stage.tile([HD, HALF], f32, tag="ogst", name="og_st")
                    nc.scalar.copy(ogst[:], pso[0:HD, :])
                    nc.sync.dma_start(
                        out=og[h * HD:(h + 1) * HD, q0:q0 + HALF], in_=ogst[:])

                # normalize P in place and stream out
                for h in range(H):
                    psb = ps_m.tile([128, HALF], f32, tag="mps", name="bcast_ps")
                    nc.tensor.matmul(psb[:], lhsT=sel[:, h, :], rhs=invdr[:],
                                     start=True, stop=True)
                    for kt in range(NKT):
                        sl = Ph[h][:, kt * HALF:(kt + 1) * HALF]
                        nc.vector.tensor_tensor(sl, sl, psb[:], MULT)
                    nc.sync.dma_start(
                        out=p_d[h, :, q0:q0 + HALF]
                        .rearrange("(kt p) q -> p kt q", p=128),
                        in_=Ph[h][:].rearrange("p (kt q) -> p kt q", kt=NKT))

                # gate * og / denom, then the output projection
                psb4 = ps_m.tile([128, HALF], f32, tag="mps", name="bcast4_ps")
                nc.tensor.matmul(psb4[:], lhsT=sel32[:], rhs=invdr[:],
                                 start=True, stop=True)
                ogh = og[:, q0:q0 + HALF]
                nc.vector.tensor_tensor(ogh, ogh, gate[:, q0:q0 + HALF], MULT)
                nc.vector.tensor_tensor(ogh, ogh, psb4[:], MULT)
                for qt in range(4):
                    qg = half * 4 + qt
                    pso2 = ps_m.tile([128, DQ], f32, tag="mps", name="out_ps")
                    nc.tensor.matmul(pso2[:], lhsT=og[:, qg * 128:(qg + 1) * 128],
                                     rhs=wo_sb[:], start=True, stop=True)
                    ot = stage.tile([128, DQ], f32, tag="ot", name="out_sb")
                    nc.scalar.copy(ot[:], pso2[:])
                    nc.sync.dma_start(out=o_d[qg * 128:(qg + 1) * 128, :], in_=ot[:])

    nc.compile()
    return nc


def _get_program():
    global _cached_nc
    if _cached_nc is None:
        _cached_nc = _build_program()
    return _cached_nc


def _host_prep(q, k, v, attn_bias, key_padding_mask, Wq, Wk, Wv, Wo, bo, Wg, bg):
    temp = np.sqrt(np.float32(HD))
    # sel[j, h*128 + m] = 1 if j == h: selects/broadcasts row h in a K=4 matmul
    sel = np.zeros((4, 4, 128), dtype=np.float32)
    for h in range(4):
        sel[h, h, :] = 1.0
    sel = sel.reshape(4, 4 * 128)
    # sel32[j, m] = 1 if m // 32 == j: row m gets head m//32's broadcast value
    sel32 = np.zeros((4, 128), dtype=np.float32)
    for h in range(4):
        sel32[h, h * 32:(h + 1) * 32] = 1.0
    in_maps = []
    for c in range(N_CORES):
        b, g = divmod(c, 2)
        cols = slice(g * CH, (g + 1) * CH)
        hs = slice(g * H, (g + 1) * H)
        biasT = np.ascontiguousarray(attn_bias[b][:, :, hs].transpose(2, 1, 0))
        biasT[:, key_padding_mask[b], :] = NEG
        in_maps.append({
            "qT": np.ascontiguousarray(q[b].T),
            "kT": np.ascontiguousarray(k[b].T),
            "vT": np.ascontiguousarray(v[b].T),
            "biasT": biasT,
            "wqT": np.ascontiguousarray(Wq[cols, :].T) / temp,
            "wkT": np.ascontiguousarray(Wk[cols, :].T),
            "wvT": np.ascontiguousarray(Wv[cols, :].T),
            "wgT": np.ascontiguousarray(Wg[cols, :].T),
            "bgCol": np.ascontiguousarray(bg[cols])[:, None],
            "woR": np.ascontiguousarray(Wo[:, cols].T),
            "sel": sel,
            "sel32": sel32,
        })
    return in_maps


def run(q, k, v, attn_bias, key_padding_mask, Wq, Wk, Wv, Wo, bo, Wg, bg,
        trace=False):
    """Returns ((out, attn), BassKernelResults)."""
    from concourse import bass_utils

    args = [np.asarray(x) for x in
            (q, k, v, attn_bias, key_padding_mask, Wq, Wk, Wv, Wo, bo, Wg, bg)]
    (q, k, v, attn_bias, key_padding_mask, Wq, Wk, Wv, Wo, bo, Wg, bg) = args
    nc = _get_program()
    in_maps = _host_prep(q, k, v, attn_bias, key_padding_mask,
                         Wq, Wk, Wv, Wo, bo, Wg, bg)
    res = bass_utils.run_bass_kernel_spmd(
        nc, in_maps, core_ids=list(range(N_CORES)), trace=trace)

    out = np.empty((BS, QL, DQ), dtype=np.float32)
    attn = np.empty((BS, QL, KL, HEADS), dtype=np.float32)
    for b in range(BS):
        out[b] = res.results[2 * b]["oPart"] + res.results[2 * b + 1]["oPart"] \
            + bo.astype(np.float32)
        for g in range(2):
            p = res.results[2 * b + g]["pOut"]          # [H, KL, QL]
            attn[b, :, :, g * H:(g + 1) * H] = p.transpose(2, 1, 0)
    return (out, attn), res


def kernel(q, k, v, attn_bias, key_padding_mask, Wq, Wk, Wv, Wo, bo, Wg, bg):
    (out, attn), _ = run(q, k, v, attn_bias, key_padding_mask,
                         Wq, Wk, Wv, Wo, bo, Wg, bg)
    return out, attn
